# revision 32
# baseline (speedup 1.0000x reference)
"""Causal self-attention (RMSNorm-QK + RoPE) Trainium2 Bass kernel.

Data-parallel over 8 NeuronCores: each core processes 8 of the 64 batches.
Per-core pipeline (all matmuls bf16, fp32 PSUM accumulation):
  x^T (host-transposed, bf16) -> QKV matmul (token layout) -> per-head
  RMSNorm (rs_q folded in by DVE, rs_k folded into exp scale) + RoPE
  (host-folded cos/sin tables incl. q_w/k_w and the 1/sqrt(D) scale)
  -> PE-transpose q,k -> scores^T = k^T.T @ q^T with causal mask
  pre-loaded into PSUM via a triangular-ones matmul -> exp on ACT ->
  AV matmul with a ones column appended to v (softmax sums for free)
  -> divide by sums -> proj matmul -> PSUM DMA'd straight to DRAM.
"""

import numpy as np
import ml_dtypes

B, T, C, H, D = 64, 512, 384, 6, 64
NCORES = 8
BPC = B // NCORES  # batches per core
EPS = 1e-6
NEG = -30000.0
BF16 = ml_dtypes.bfloat16

TM = T // 128   # 4 token tiles per batch
KC = C // 128   # 3 channel chunks
HPC = 128 // D  # 2 heads per channel chunk


def _rope_tables(w, scale):
    """cos/sin tables with rmsnorm weight + scale folded in, shift-form sign."""
    d = D
    inv_freq = 1.0 / (10000.0 ** (np.arange(0, d, 2, dtype=np.float64) / d))
    freqs = np.arange(T, dtype=np.float64)[:, None] * inv_freq[None, :]
    emb = np.concatenate([freqs, freqs], axis=-1)  # [T, D]
    cos, sin = np.cos(emb), np.sin(emb)
    # out[d] = x[d]*cos[d] + shift(x)[d]*sin_s[d]; shift moves by +-32 within head
    sin_s = sin.copy()
    sin_s[:, : d // 2] *= -1.0
    w = np.asarray(w, dtype=np.float64)
    w_shift = np.concatenate([w[d // 2:], w[: d // 2]])
    cos_e = cos * w[None, :] * scale
    sin_e = sin_s * w_shift[None, :] * scale
    # tile across H heads -> [T, C]
    cos_full = np.tile(cos_e, (1, H))
    sin_full = np.tile(sin_e, (1, H))
    return cos_full.astype(BF16), sin_full.astype(BF16)


def _build(nc):
    import concourse.mybir as mybir
    from concourse.tile import TileContext

    f32 = mybir.dt.float32
    bf = mybir.dt.bfloat16
    AX = mybir.AxisListType.X
    MUL = mybir.AluOpType.mult
    ADD = mybir.AluOpType.add
    ACT = mybir.ActivationFunctionType

    xT = nc.declare_dram_parameter("xT", [BPC, C, T], bf, isOutput=False)
    wqkv = nc.declare_dram_parameter("wqkv", [C, 3 * C], bf, isOutput=False)
    wproj = nc.declare_dram_parameter("wproj", [C, C], bf, isOutput=False)
    cosq = nc.declare_dram_parameter("cosq", [T, C], bf, isOutput=False)
    sinq = nc.declare_dram_parameter("sinq", [T, C], bf, isOutput=False)
    cosk = nc.declare_dram_parameter("cosk", [T, C], bf, isOutput=False)
    sink = nc.declare_dram_parameter("sink", [T, C], bf, isOutput=False)
    ident = nc.declare_dram_parameter("ident", [128, 128], bf, isOutput=False)
    triu1 = nc.declare_dram_parameter("triu1", [128, 128], bf, isOutput=False)
    negeye = nc.declare_dram_parameter("negeye", [128, 128], bf, isOutput=False)
    sel3 = nc.declare_dram_parameter("sel3", [3, 3 * D], bf, isOutput=False)
    out = nc.declare_dram_parameter("out", [BPC, T, C], f32, isOutput=True)

    with TileContext(nc) as tc:
        with (
            tc.tile_pool(name="const", bufs=1) as const,
            tc.tile_pool(name="xt", bufs=3) as xt_pool,
            tc.tile_pool(name="qkv", bufs=2) as qkv_pool,
            tc.tile_pool(name="tmp", bufs=2) as tmp_pool,
            tc.tile_pool(name="stat", bufs=2) as stat_pool,
            tc.tile_pool(name="tp", bufs=2) as tp_pool,
            tc.tile_pool(name="probs", bufs=6) as probs_pool,
            tc.tile_pool(name="ytile", bufs=2) as y_pool,
            tc.tile_pool(name="sums", bufs=2) as sums_pool,
            tc.tile_pool(name="ps_qkv", bufs=1, space="PSUM") as ps_qkv,
            tc.tile_pool(name="ps_tp", bufs=1, space="PSUM") as ps_tp,
            tc.tile_pool(name="ps_sc", bufs=2, space="PSUM") as ps_sc,
            tc.tile_pool(name="ps_av", bufs=1, space="PSUM") as ps_av,
            tc.tile_pool(name="ps_pr", bufs=1, space="PSUM") as ps_pr,
        ):
            # ---- constants / weights resident in SBUF ----
            w_sb = const.tile([128, KC, 3 * C], bf)
            nc.sync.dma_start(out=w_sb, in_=wqkv.rearrange("(k p) n -> p k n", p=128))
            wp_sb = const.tile([128, KC, C], bf)
            nc.sync.dma_start(out=wp_sb, in_=wproj.rearrange("(k p) n -> p k n", p=128))
            rope_sb = {}
            for nm, t in (("cosq", cosq), ("sinq", sinq), ("cosk", cosk), ("sink", sink)):
                rt = const.tile([128, TM, C], bf, tag=nm)
                nc.scalar.dma_start(out=rt, in_=t.rearrange("(t p) c -> p t c", p=128))
                rope_sb[nm] = rt
            id_sb = const.tile([128, 128], bf, tag="ident")
            nc.sync.dma_start(out=id_sb[:], in_=ident[:])
            tri_sb = const.tile([128, 128], bf, tag="triu1")
            nc.sync.dma_start(out=tri_sb[:], in_=triu1[:])
            neg_sb = const.tile([128, 128], bf, tag="negeye")
            nc.sync.dma_start(out=neg_sb[:], in_=negeye[:])
            sel_sb = const.tile([3, 3 * D], bf, tag="sel3")
            nc.scalar.dma_start(out=sel_sb[:], in_=sel3[:])
            eps_sb = const.tile([128, 1], f32, tag="eps")
            nc.vector.memset(eps_sb, EPS * D)

            for b in range(BPC):
                # ---- load x^T for this batch ----
                xt = xt_pool.tile([128, KC, T], bf)
                nc.sync.dma_start(
                    out=xt, in_=xT[b].rearrange("(k p) t -> p k t", p=128)
                )

                # ---- QKV matmul: token layout [tok, 3C] ----
                q_sb = qkv_pool.tile([128, TM, C], bf, tag="q")
                k_sb = qkv_pool.tile([128, TM, C], bf, tag="k")
                v_sb = qkv_pool.tile([128, TM, H, D + 1], bf, tag="v")
                nc.gpsimd.memset(v_sb[:, :, :, D], 1.0)  # ones col for sums
                for m in range(TM):
                    for sec, dst in ((0, q_sb), (1, k_sb), (2, v_sb)):
                        ps = ps_qkv.tile([128, C], f32, tag="qkv")
                        for kc in range(KC):
                            nc.tensor.matmul(
                                ps,
                                xt[:, kc, m * 128:(m + 1) * 128],
                                w_sb[:, kc, sec * C:(sec + 1) * C],
                                start=(kc == 0),
                                stop=(kc == KC - 1),
                            )
                        if sec == 2:
                            nc.vector.tensor_copy(
                                v_sb[:, m, :, 0:D], ps.rearrange("p (h d) -> p h d", h=H)
                            )
                        else:
                            nc.scalar.copy(dst[:, m, :], ps)

                # ---- RMSNorm stats (per token, per head) ----
                rs = {}
                for nm, src in (("q", q_sb), ("k", k_sb)):
                    sq = tmp_pool.tile([128, TM, C], bf, tag="sq")
                    nc.vector.tensor_mul(sq, src, src)
                    ssum = stat_pool.tile([128, TM, H], f32, tag="ss" + nm)
                    nc.vector.tensor_reduce(
                        ssum, sq.rearrange("p t (h d) -> p t h d", h=H), axis=AX, op=ADD
                    )
                    lg = stat_pool.tile([128, TM, H], f32, tag="lg" + nm)
                    nc.scalar.activation(lg, ssum, ACT.Ln, bias=eps_sb[:], scale=1.0)
                    r = stat_pool.tile([128, TM, H], f32, tag="rs" + nm)
                    # rs = 1/sqrt(ssum/D + eps) = sqrt(D) * exp(-0.5*ln(ssum + eps*D))
                    nc.scalar.activation(r, lg, ACT.Exp, scale=-0.5)
                    rs[nm] = r

                # ---- RoPE (+ rs_q fold) ----
                roped = {}
                for nm, src in (("q", q_sb), ("k", k_sb)):
                    sh = tmp_pool.tile([128, TM, H, 2, D // 2], bf, tag="sh")
                    s4 = src.rearrange("p t (h s d) -> p t h s d", h=H, s=2)
                    nc.gpsimd.tensor_copy(sh[:, :, :, 0, :], s4[:, :, :, 1, :])
                    nc.gpsimd.tensor_copy(sh[:, :, :, 1, :], s4[:, :, :, 0, :])
                    m1 = tmp_pool.tile([128, TM, C], bf, tag="m1")
                    nc.vector.tensor_mul(m1, src, rope_sb["cos" + nm])
                    m2 = tmp_pool.tile([128, TM, C], bf, tag="m2")
                    nc.vector.tensor_mul(
                        m2, sh.rearrange("p t h s d -> p t (h s d)"), rope_sb["sin" + nm]
                    )
                    ro = tmp_pool.tile([128, TM, C], bf, tag="ro" + nm)
                    if nm == "q":
                        # (m1 + m2) * rs_q  via scalar_tensor_tensor? needs per-head
                        # scalar -> do TT add then TT mul with broadcast rs
                        nc.gpsimd.tensor_add(ro, m1, m2)
                        roq = tmp_pool.tile([128, TM, C], bf, tag="roq")
                        nc.vector.tensor_mul(
                            roq.rearrange("p t (h d) -> p t h d", h=H),
                            ro.rearrange("p t (h d) -> p t h d", h=H),
                            rs["q"][:, :, :, None].broadcast_to((128, TM, H, D)),
                        )
                        roped[nm] = roq
                    else:
                        nc.gpsimd.tensor_add(ro, m1, m2)
                        roped[nm] = ro

                # ---- transpose q,k -> [C, T] layout ----
                qkT = {}
                for nm in ("q", "k"):
                    dst = tp_pool.tile([128, KC, T], bf, tag="T" + nm)
                    for kc in range(KC):
                        pst = ps_tp.tile([128, T], bf, tag="tp")
                        for m in range(TM):
                            nc.tensor.transpose(
                                pst[:, m * 128:(m + 1) * 128],
                                roped[nm][:, m, kc * 128:(kc + 1) * 128],
                                id_sb,
                            )
                        nc.vector.tensor_copy(dst[:, kc, :], pst)
                    qkT[nm] = dst

                # ---- attention, heads in groups of 3 ----
                yall = y_pool.tile([128, KC, T], bf, tag="yall")
                for g in range(2):
                    yps = ps_av.tile([D + 1, 3, T], f32, tag="av")
                    for hh in range(3):
                        h = g * 3 + hh
                        qTh = qkT["q"][D * (h % HPC):D * (h % HPC) + D, h // HPC, :]
                        kTh = qkT["k"][D * (h % HPC):D * (h % HPC) + D, h // HPC, :]
                        for j in range(TM):
                            nj = T - j * 128
                            sc = ps_sc.tile([128, T], f32, tag="sc")
                            nc.tensor.matmul(
                                sc[:, 0:nj],
                                kTh[:, j * 128:(j + 1) * 128],
                                qTh[:, j * 128:T],
                                start=True,
                                stop=False,
                            )
                            # causal mask for diagonal block, added by matmul
                            nc.tensor.matmul(
                                sc[:, 0:128], tri_sb, neg_sb, start=False, stop=True
                            )
                            pr = probs_pool.tile([128, T], bf, tag="pr")
                            nc.scalar.activation(
                                pr[:, 0:nj], sc[:, 0:nj], ACT.Exp,
                                scale=rs["k"][:, j, h:h + 1],
                            )
                            nc.tensor.matmul(
                                yps[:, hh, j * 128:T],
                                v_sb[:, j, h, :],
                                pr[:, 0:nj],
                                start=(j == 0),
                                stop=(j == TM - 1),
                            )
                    # copy group to SBUF (bf16), pull sums row out, divide
                    yc = y_pool.tile([D + 1, 3, T], bf, tag="yc")
                    nc.vector.tensor_copy(yc, yps)
                    sums3 = sums_pool.tile([3, T], bf, tag="sums")
                    nc.sync.dma_start(out=sums3, in_=yc[D:D + 1, :, :])
                    inv3 = sums_pool.tile([3, T], f32, tag="invs")
                    nc.vector.reciprocal(inv3, sums3)
                    inv3b = sums_pool.tile([3, T], bf, tag="invb")
                    nc.vector.tensor_copy(inv3b, inv3)
                    for hh in range(3):
                        h = g * 3 + hh
                        # broadcast inv row hh to 64 partitions via selector
                        # matmul (scores-pool slot reused as scratch)
                        ibc_full = ps_sc.tile([128, T], f32, tag="sc")
                        ibc = ibc_full[0:D, :]
                        nc.tensor.matmul(
                            ibc,
                            sel_sb[:, hh * D:(hh + 1) * D],
                            inv3b,
                            start=True,
                            stop=True,
                        )
                        nc.vector.tensor_mul(
                            yall[D * (h % HPC):D * (h % HPC) + D, h // HPC, :],
                            yc[0:D, hh, :],
                            ibc,
                        )

                # ---- output projection; DMA PSUM straight to DRAM ----
                for m in range(TM):
                    pp = ps_pr.tile([128, C], f32, tag="proj")
                    for kc in range(KC):
                        nc.tensor.matmul(
                            pp,
                            yall[:, kc, m * 128:(m + 1) * 128],
                            wp_sb[:, kc, :],
                            start=(kc == 0),
                            stop=(kc == KC - 1),
                        )
                    osb = y_pool.tile([128, C], f32, tag="osb")
                    nc.scalar.copy(osb, pp)
                    nc.sync.dma_start(
                        out=out[b, m * 128:(m + 1) * 128, :], in_=osb
                    )
    return nc


_CACHE = {}


def _get_nc():
    if "nc" not in _CACHE:
        from concourse import bacc
        nc = _build(bacc.Bacc())
        nc.compile()
        _CACHE["nc"] = nc
    return _CACHE["nc"]


def kernel(x, w_qkv, w_proj, q_w, k_w):
    from concourse.bass_utils import run_bass_kernel_spmd

    x = np.asarray(x)
    # rs_hat = rs_true/sqrt(D) for both q and k; fold D/sqrt(D)=sqrt(D) here
    cq, sq_ = _rope_tables(np.asarray(q_w), np.sqrt(D))
    ck, sk_ = _rope_tables(np.asarray(k_w), 1.0)
    wqkv_b = np.asarray(w_qkv).astype(BF16)
    wproj_b = np.asarray(w_proj).astype(BF16)
    ident = np.eye(128, dtype=np.float32).astype(BF16)
    triu1 = np.triu(np.ones((128, 128), dtype=np.float32), 1).astype(BF16)
    negeye = (np.eye(128, dtype=np.float32) * NEG).astype(BF16)
    sel3 = np.kron(np.eye(3, dtype=np.float32), np.ones((1, D), np.float32)).astype(BF16)

    in_maps = []
    for c in range(NCORES):
        xs = x[c * BPC:(c + 1) * BPC].astype(BF16)  # [BPC, T, C]
        xs_t = np.ascontiguousarray(xs.transpose(0, 2, 1))  # [BPC, C, T]
        in_maps.append({
            "xT": xs_t, "wqkv": wqkv_b, "wproj": wproj_b,
            "cosq": cq, "sinq": sq_, "cosk": ck, "sink": sk_,
            "ident": ident, "triu1": triu1, "negeye": negeye, "sel3": sel3,
        })

    nc = _get_nc()
    res = run_bass_kernel_spmd(nc, in_maps, core_ids=list(range(NCORES)))
    return np.concatenate([r["out"] for r in res.results], axis=0)
